# revision 14
# baseline (speedup 1.0000x reference)
"""Multi-head self-attention 2d kernel for 8 trn2 NeuronCores.

Sharding: data-parallel over batch B=16 -> 2 batches per core.
Per-core Bass/Tile kernel computes the full attention block for its 2 batches.

Dataflow (per batch, per core):
  xf [C=512 part, N=1024 free]  (C on partitions, 4 tiles of 128), f32r
  q = wq@xf + bq       -> [C, N] bf16   (lhsT = wqT tiles, f32r matmul)
  k = wk@xf + bk       -> [C, N] bf16
  vT = xf.T@wvT + bv   -> v_ext [N, 65, 8] bf16 (ones row at [., 64, .])
  per head h (bf16 matmuls, fp32 psum):
    eT[j, i] = k_h.T @ q_h        (K=64; two heads packed in PE row groups)
    expT = exp(SCALE * eT) bf16   (ACT, no max subtraction; |SCALE*e| < 8)
    out_u[0:65, i] = v_ext_h.T @ expT   (accumulate over j; row 64 = denom)
    r = 1/denom broadcast over 64 partitions via K=1 matmul with ones
    out_norm[h*64:(h+1)*64, :] = out_u[0:64] * r   (f32r)
  y = gamma*(wo@out_norm + bo) + x    (f32r matmul, f32 residual)
"""

import sys

for _p in ("/opt/trn_rl_repo",):
    if _p not in sys.path:
        sys.path.insert(0, _p)

import numpy as np
import ml_dtypes

import concourse.bass as bass
from concourse import bacc
import concourse.mybir as mybir
import concourse.tile as tile
from concourse.bass_utils import run_bass_kernel_spmd

F32 = mybir.dt.float32
F32R = mybir.dt.float32r
BF16 = mybir.dt.bfloat16
AF = mybir.ActivationFunctionType
ALU = mybir.AluOpType

C = 512
N = 1024
HEADS = 8
HD = C // HEADS  # 64
SCALE = HD ** -0.5
CT = C // 128  # 4 channel tiles
NT = N // 128  # 8 spatial tiles
NCH = N // 512  # 2 free-dim chunks
BPC = 2  # batches per core
NCORES = 8


def _r(ap):
    return ap.bitcast(F32R)


def build_program():
    nc = bacc.Bacc(trn_type="TRN2", target_bir_lowering=False, debug=False,
                   num_devices=NCORES)

    x2 = nc.dram_tensor("x2", [BPC, C, N], F32R, kind="ExternalInput").ap()
    wT = {
        name: nc.dram_tensor(name, [C, C], F32R, kind="ExternalInput").ap()
        for name in ("wqT", "wkT", "wvT", "woT")
    }
    bq_r = nc.dram_tensor("bq_r", [128, CT], F32, kind="ExternalInput").ap()
    bk_r = nc.dram_tensor("bk_r", [128, CT], F32, kind="ExternalInput").ap()
    bo_r = nc.dram_tensor("bo_r", [128, CT], F32, kind="ExternalInput").ap()
    bv = nc.dram_tensor("bv", [C], F32, kind="ExternalInput").ap()
    gamma = nc.dram_tensor("gamma", [1], F32, kind="ExternalInput").ap()
    ones64 = nc.dram_tensor("ones64", [HD], F32R, kind="ExternalInput").ap()
    ones64h = nc.dram_tensor("ones64h", [HD], BF16, kind="ExternalInput").ap()
    y2 = nc.dram_tensor("y2", [BPC, C, N], F32, kind="ExternalOutput").ap()

    with tile.TileContext(nc) as tc:
        with (
            tc.tile_pool(name="sb", bufs=1) as sb,
            tc.tile_pool(name="ps", bufs=1, space="PSUM") as ps,
        ):
            # ---- persistent weights / biases ----
            w_sb = {}
            _dmae = [nc.scalar, nc.gpsimd, nc.sync]
            _di = 0
            for name in ("wqT", "wkT", "wvT", "woT"):
                tiles = []
                for kc in range(CT):
                    t = sb.tile([128, C], F32R, tag=f"{name}{kc}")
                    _dmae[_di % 3].dma_start(
                        out=t, in_=wT[name][kc * 128:(kc + 1) * 128, :])
                    _di += 1
                    tiles.append(t)
                w_sb[name] = tiles

            bq_sb = sb.tile([128, CT], F32, tag="bq")
            nc.gpsimd.dma_start(out=bq_sb, in_=bq_r)
            bk_sb = sb.tile([128, CT], F32, tag="bk")
            nc.gpsimd.dma_start(out=bk_sb, in_=bk_r)
            bo_sb = sb.tile([128, CT], F32, tag="bo")
            nc.gpsimd.dma_start(out=bo_sb, in_=bo_r)
            bv_bc = sb.tile([128, C], F32, tag="bv")
            nc.gpsimd.dma_start(
                out=bv_bc,
                in_=bass.AP(tensor=bv.tensor, offset=bv.offset,
                            ap=[[0, 128]] + list(bv.ap)),
            )
            gam_sb = sb.tile([128, 1], F32, tag="gam")
            nc.gpsimd.dma_start(
                out=gam_sb,
                in_=bass.AP(tensor=gamma.tensor, offset=gamma.offset,
                            ap=[[0, 128]] + list(gamma.ap)),
            )
            ones1 = sb.tile([1, HD], F32R, tag="ones1")
            nc.gpsimd.dma_start(
                out=ones1,
                in_=bass.AP(tensor=ones64.tensor, offset=ones64.offset,
                            ap=[[0, 1]] + list(ones64.ap)))
            # v_ext tiles persist across batches (bufs=2 for batch overlap);
            # layout [128 j, 65 (d|one), 8 h]: ones row is contiguous 32B.
            vext_tiles = {}
            for bb in range(BPC):
                for nt in range(NT):
                    t = sb.tile([128, HD + 1, HEADS], BF16, tag=f"v{nt}",
                                name=f"vext{bb}_{nt}", bufs=2)
                    nc.gpsimd.dma_start(
                        out=t[:, HD, :],
                        in_=bass.AP(tensor=ones64h.tensor, offset=ones64h.offset,
                                    ap=[[0, 128], [1, HEADS]]))
                    vext_tiles[(bb, nt)] = t

            for b in range(BPC):
                # ---- load x ----
                xf = []
                for ct in range(CT):
                    t = sb.tile([128, N], F32R, tag=f"xf{ct}", bufs=2)
                    nc.sync.dma_start(out=t, in_=x2[b, ct * 128:(ct + 1) * 128, :])
                    xf.append(t)

                # ---- Q / K projections (f32r matmul -> bf16 tiles) ----
                q_sb, k_sb = [], []
                for wname, bias_sb, dst in (("wqT", bq_sb, q_sb),
                                            ("wkT", bk_sb, k_sb)):
                    for ot in range(CT):
                        t = sb.tile([128, N], F32R, tag=f"{wname}o{ot}", bufs=2)
                        for nch in range(NCH):
                            p = ps.tile([128, 512], F32, tag="pq", bufs=2)
                            for kc in range(CT):
                                nc.tensor.matmul(
                                    p,
                                    lhsT=w_sb[wname][kc][:, ot * 128:(ot + 1) * 128],
                                    rhs=_r(xf[kc][:, nch * 512:(nch + 1) * 512]),
                                    start=(kc == 0), stop=(kc == CT - 1),
                                )
                            nc.vector.tensor_scalar_add(
                                t[:, nch * 512:(nch + 1) * 512], p,
                                bias_sb[:, ot:ot + 1])
                        dst.append(t)

                # ---- V projection, transposed -> bf16 v_ext ----
                v_ext = [vext_tiles[(b, nt)] for nt in range(NT)]
                for nt in range(NT):
                    p = ps.tile([128, 512], F32, tag="pq", bufs=2)
                    for kc in range(CT):
                        nc.tensor.matmul(
                            p,
                            lhsT=_r(xf[kc][:, nt * 128:(nt + 1) * 128]),
                            rhs=_r(w_sb["wvT"][kc]),
                            start=(kc == 0), stop=(kc == CT - 1),
                        )
                    nc.vector.tensor_tensor(
                        v_ext[nt][:, 0:HD, :],
                        p.rearrange("p (h d) -> p d h", h=HEADS),
                        bv_bc.rearrange("p (h d) -> p d h", h=HEADS),
                        ALU.add,
                    )

                # ---- attention (bf16 matmuls) ----
                on_sb = [sb.tile([128, N], F32R, tag=f"on{ct}",
                                 name=f"on{b}_{ct}", bufs=1)
                         for ct in range(CT)]
                for hp in range(HEADS // 2):
                    expT = [[], []]
                    for jt in range(NT):
                        pe_pair = [ps.tile([128, N], F32, tag="pe", bufs=2,
                                           name=f"pe{b}_{hp}_{jt}_{hh}")
                                   for hh in range(2)]
                        for ic in range(NCH):
                            for hh in range(2):
                                nc.tensor.matmul(
                                    pe_pair[hh][:, ic * 512:(ic + 1) * 512],
                                    lhsT=k_sb[hp][hh * 64:(hh + 1) * 64,
                                                  jt * 128:(jt + 1) * 128],
                                    rhs=q_sb[hp][hh * 64:(hh + 1) * 64,
                                                 ic * 512:(ic + 1) * 512],
                                    start=True, stop=True,
                                )
                        for hh in range(2):
                            e = sb.tile([128, N], BF16, tag="exp", bufs=8)
                            nc.scalar.activation(e, pe_pair[hh], AF.Exp,
                                                 scale=SCALE)
                            expT[hh].append(e)
                    for hh in range(2):
                        h = 2 * hp + hh
                        ct, half = divmod(h, 2)
                        pus = [ps.tile([128, 512], F32, tag="pu", bufs=2,
                                       name=f"pu{b}_{h}_{ic}")
                               for ic in range(NCH)]
                        for jt in range(NT):
                            for ic in range(NCH):
                                nc.tensor.matmul(
                                    pus[ic][0:HD + 1, :],
                                    lhsT=v_ext[jt][:, :, h],
                                    rhs=expT[hh][jt][:, ic * 512:(ic + 1) * 512],
                                    start=(jt == 0), stop=(jt == NT - 1),
                                )
                        for ic in range(NCH):
                            pu = pus[ic]
                            den = sb.tile([1, 512], F32R, tag="den", bufs=2)
                            nc.vector.tensor_copy(den, pu[HD:HD + 1, :])
                            rb = ps.tile([HD, 512], F32, tag="pq", bufs=2)
                            nc.tensor.matmul(rb, lhsT=_r(ones1), rhs=_r(den),
                                             start=True, stop=True)
                            r_sb = sb.tile([HD, 512], F32, tag="rsb", bufs=2)
                            nc.vector.reciprocal_approx_fast(out=r_sb, in_=rb)
                            nc.vector.tensor_tensor(
                                on_sb[ct][half * 64:(half + 1) * 64,
                                          ic * 512:(ic + 1) * 512],
                                pu[0:HD, :], r_sb, ALU.mult)

                # ---- out projection + residual + store ----
                for ot in range(CT):
                    for nch in range(NCH):
                        p = ps.tile([128, 512], F32, tag="pq", bufs=2)
                        for ctt in range(CT):
                            nc.tensor.matmul(
                                p,
                                lhsT=w_sb["woT"][ctt][:, ot * 128:(ot + 1) * 128],
                                rhs=on_sb[ctt][:, nch * 512:(nch + 1) * 512],
                                start=(ctt == 0), stop=(ctt == CT - 1),
                            )
                        yt = sb.tile([128, 512], F32, tag="y", bufs=4)
                        nc.vector.tensor_scalar(
                            yt, p, bo_sb[:, ot:ot + 1], gam_sb[:, 0:1],
                            ALU.add, ALU.mult)
                        nc.vector.tensor_tensor(
                            yt, yt,
                            xf[ot][:, nch * 512:(nch + 1) * 512].bitcast(F32),
                            ALU.add)
                        nc.gpsimd.dma_start(
                            out=y2[b, ot * 128:(ot + 1) * 128,
                                   nch * 512:(nch + 1) * 512],
                            in_=yt)
    nc.compile()
    return nc


_PROGRAM = None


def _get_program():
    global _PROGRAM
    if _PROGRAM is None:
        _PROGRAM = build_program()
    return _PROGRAM


def kernel(**inputs):
    x = np.ascontiguousarray(inputs["x"], dtype=np.float32)
    B, c, H, W = x.shape
    assert (c, H * W) == (C, N)
    xr = x.reshape(B, C, N)

    wqT = np.ascontiguousarray(inputs["wq"].T.astype(np.float32))
    wkT = np.ascontiguousarray(inputs["wk"].T.astype(np.float32))
    wvT = np.ascontiguousarray(inputs["wv"].T.astype(np.float32))
    woT = np.ascontiguousarray(inputs["wo"].T.astype(np.float32))
    bq_r = np.ascontiguousarray(inputs["bq"].astype(np.float32).reshape(CT, 128).T)
    bk_r = np.ascontiguousarray(inputs["bk"].astype(np.float32).reshape(CT, 128).T)
    bo_r = np.ascontiguousarray(inputs["bo"].astype(np.float32).reshape(CT, 128).T)
    bv = np.ascontiguousarray(inputs["bv"].astype(np.float32))
    gamma = np.ascontiguousarray(inputs["gamma"].astype(np.float32))

    shared = dict(wqT=wqT, wkT=wkT, wvT=wvT, woT=woT,
                  bq_r=bq_r, bk_r=bk_r, bo_r=bo_r, bv=bv, gamma=gamma,
                  ones64=np.ones(HD, dtype=np.float32),
                  ones64h=np.ones(HD, dtype=ml_dtypes.bfloat16))
    in_maps = []
    for core in range(NCORES):
        m = dict(shared)
        m["x2"] = np.ascontiguousarray(xr[core * BPC:(core + 1) * BPC])
        in_maps.append(m)

    nc = _get_program()
    res = run_bass_kernel_spmd(nc, in_maps, list(range(NCORES)))
    y = np.concatenate([res.results[i]["y2"] for i in range(NCORES)], axis=0)
    return y.reshape(B, C, H, W).astype(np.float32)


if __name__ == "__main__":
    rng = np.random.default_rng(0)
    ins = {
        "x": rng.standard_normal((16, C, 32, 32), dtype=np.float32),
        "wq": rng.standard_normal((C, C), dtype=np.float32) / 23,
        "bq": rng.standard_normal((C,), dtype=np.float32) / 23,
        "wk": rng.standard_normal((C, C), dtype=np.float32) / 23,
        "bk": rng.standard_normal((C,), dtype=np.float32) / 23,
        "wv": rng.standard_normal((C, C), dtype=np.float32) / 23,
        "bv": rng.standard_normal((C,), dtype=np.float32) / 23,
        "wo": rng.standard_normal((C, C), dtype=np.float32) / 23,
        "bo": rng.standard_normal((C,), dtype=np.float32) / 23,
        "gamma": np.full((1,), 0.1, dtype=np.float32),
    }
    out = kernel(**ins)
    print("kernel ran, out shape", out.shape)


# revision 17
# speedup vs baseline: 1.0643x; 1.0643x over previous
"""Multi-head self-attention 2d kernel for 8 trn2 NeuronCores.

Sharding: data-parallel over batch B=16 -> 2 batches per core.

Per-core dataflow (per batch):
  xf [C=512 part, N=1024 free] f32r
  q = wq@xf + bq  -> [C, N] f32r     (f32r matmuls, fp32 psum)
  k = wk@xf + bk  -> [C, N] f32r
  vT = xf.T@wvT + bv -> v_ext [N, 65, 8] bf16 (ones row at [., 64, .])
  per head h:
    eT[j, i] = k_h.T @ q_h           (f32r, K=64, row-group packed pairs)
    expT = exp(SCALE * eT) bf16      (ACT; no max subtraction, |SCALE*e| < 8)
    out_u[0:65, i] = v_ext_h.T @ expT  (bf16; accumulate over j; row 64=denom)
    r = 1/denom broadcast over 64 partitions via K=1 matmul with ones
    out_norm = out_u[0:64] * r       (f32r)
  y = gamma*(wo@out_norm + bo) + x   (f32r matmul, f32 residual)

Program order interleaves batch1 projections into batch0's attention pairs
(and batch0's out-projection into batch1's first pairs) so the scalar engine
(exp) never idles at phase boundaries.
"""

import sys

for _p in ("/opt/trn_rl_repo",):
    if _p not in sys.path:
        sys.path.insert(0, _p)

import numpy as np
import ml_dtypes

import concourse.bass as bass
from concourse import bacc
import concourse.mybir as mybir
import concourse.tile as tile
from concourse.bass_utils import run_bass_kernel_spmd

F32 = mybir.dt.float32
F32R = mybir.dt.float32r
BF16 = mybir.dt.bfloat16
AF = mybir.ActivationFunctionType
ALU = mybir.AluOpType

C = 512
N = 1024
HEADS = 8
HD = C // HEADS  # 64
SCALE = HD ** -0.5
CT = C // 128  # 4 channel tiles
NT = N // 128  # 8 spatial tiles
NCH = N // 512  # 2 free-dim chunks
BPC = 2  # batches per core
NCORES = 8


def _r(ap):
    return ap.bitcast(F32R)


def build_program():
    nc = bacc.Bacc(trn_type="TRN2", target_bir_lowering=False, debug=False,
                   num_devices=NCORES)

    x2 = nc.dram_tensor("x2", [BPC, C, N], F32R, kind="ExternalInput").ap()
    wT = {
        name: nc.dram_tensor(name, [C, C], F32R, kind="ExternalInput").ap()
        for name in ("wqT", "wkT", "wvT", "woT")
    }
    bq_r = nc.dram_tensor("bq_r", [128, CT], F32, kind="ExternalInput").ap()
    bk_r = nc.dram_tensor("bk_r", [128, CT], F32, kind="ExternalInput").ap()
    bo_r = nc.dram_tensor("bo_r", [128, CT], F32, kind="ExternalInput").ap()
    bv = nc.dram_tensor("bv", [C], F32, kind="ExternalInput").ap()
    gamma = nc.dram_tensor("gamma", [1], F32, kind="ExternalInput").ap()
    ones64 = nc.dram_tensor("ones64", [HD], F32R, kind="ExternalInput").ap()
    ones64h = nc.dram_tensor("ones64h", [HD], BF16, kind="ExternalInput").ap()
    y2 = nc.dram_tensor("y2", [BPC, C, N], F32, kind="ExternalOutput").ap()

    with tile.TileContext(nc) as tc:
        with (
            tc.tile_pool(name="sb", bufs=1) as sb,
            tc.tile_pool(name="ps", bufs=1, space="PSUM") as ps,
        ):
            st = {"xf": {}, "vext": {}, "on": {},
                  "q": {0: [None] * CT, 1: [None] * CT},
                  "k": {0: [None] * CT, 1: [None] * CT}}

            def load_x(b):
                tiles = []
                for ct in range(CT):
                    t = sb.tile([128, N], F32R, tag=f"xf{ct}", bufs=2,
                                name=f"xf{b}_{ct}")
                    nc.sync.dma_start(out=t,
                                      in_=x2[b, ct * 128:(ct + 1) * 128, :])
                    tiles.append(t)
                st["xf"][b] = tiles

            load_x(0)

            w_sb = {}
            _dmae = [nc.scalar, nc.gpsimd, nc.sync]
            _di = 0
            for name in ("wqT", "wkT", "wvT", "woT"):
                tiles = []
                for kc in range(CT):
                    t = sb.tile([128, C], F32R, tag=f"{name}{kc}")
                    _dmae[_di % 3].dma_start(
                        out=t, in_=wT[name][kc * 128:(kc + 1) * 128, :])
                    _di += 1
                    tiles.append(t)
                w_sb[name] = tiles

            bq_sb = sb.tile([128, CT], F32, tag="bq")
            nc.gpsimd.dma_start(out=bq_sb, in_=bq_r)
            bk_sb = sb.tile([128, CT], F32, tag="bk")
            nc.gpsimd.dma_start(out=bk_sb, in_=bk_r)
            bo_sb = sb.tile([128, CT], F32, tag="bo")
            nc.gpsimd.dma_start(out=bo_sb, in_=bo_r)
            bv_bc = sb.tile([128, C], F32, tag="bv")
            nc.gpsimd.dma_start(
                out=bv_bc,
                in_=bass.AP(tensor=bv.tensor, offset=bv.offset,
                            ap=[[0, 128]] + list(bv.ap)))
            gam_sb = sb.tile([128, 1], F32, tag="gam")
            nc.gpsimd.dma_start(
                out=gam_sb,
                in_=bass.AP(tensor=gamma.tensor, offset=gamma.offset,
                            ap=[[0, 128]] + list(gamma.ap)))
            ones1 = sb.tile([1, HD], F32R, tag="ones1")
            nc.gpsimd.dma_start(
                out=ones1,
                in_=bass.AP(tensor=ones64.tensor, offset=ones64.offset,
                            ap=[[0, 1]] + list(ones64.ap)))

            # v_ext tiles: [128 j, 65 (d|one), 8 h]; ones row loaded once
            # per slot (contiguous 32B per partition).
            for bb in range(BPC):
                for nt in range(NT):
                    t = sb.tile([128, HD + 1, HEADS], BF16, tag=f"v{nt}",
                                name=f"vext{bb}_{nt}", bufs=2)
                    nc.gpsimd.dma_start(
                        out=t[:, HD, :],
                        in_=bass.AP(tensor=ones64h.tensor,
                                    offset=ones64h.offset,
                                    ap=[[0, 128], [1, HEADS]]))
                    st["vext"][(bb, nt)] = t

            def proj_qk_group(b, wname, ot):
                bias_sb, dstkey = (bq_sb, "q") if wname == "wqT" else (bk_sb, "k")
                t = sb.tile([128, N], F32R, tag=f"{wname}o{ot}", bufs=2,
                            name=f"{dstkey}{b}_{ot}")
                for nch in range(NCH):
                    p = ps.tile([128, 512], F32, tag="pq", bufs=2,
                                name=f"pj{b}{wname}{ot}{nch}")
                    for kc in range(CT):
                        nc.tensor.matmul(
                            p,
                            lhsT=w_sb[wname][kc][:, ot * 128:(ot + 1) * 128],
                            rhs=_r(st["xf"][b][kc][:, nch * 512:(nch + 1) * 512]),
                            start=(kc == 0), stop=(kc == CT - 1),
                        )
                    nc.vector.tensor_scalar_add(
                        t[:, nch * 512:(nch + 1) * 512], p, bias_sb[:, ot:ot + 1])
                st[dstkey][b][ot] = t

            def proj_v_group(b, nt):
                p = ps.tile([128, 512], F32, tag="pq", bufs=2,
                            name=f"pv{b}{nt}")
                for kc in range(CT):
                    nc.tensor.matmul(
                        p,
                        lhsT=_r(st["xf"][b][kc][:, nt * 128:(nt + 1) * 128]),
                        rhs=_r(w_sb["wvT"][kc]),
                        start=(kc == 0), stop=(kc == CT - 1),
                    )
                nc.vector.tensor_tensor(
                    st["vext"][(b, nt)][:, 0:HD, :],
                    p.rearrange("p (h d) -> p d h", h=HEADS),
                    bv_bc.rearrange("p (h d) -> p d h", h=HEADS),
                    ALU.add,
                )

            def alloc_on(b):
                st["on"][b] = [sb.tile([128, N], F32R, tag=f"on{ct}",
                                       name=f"on{b}_{ct}", bufs=2)
                               for ct in range(CT)]

            def attention_pair(b, hp):
                q_sb, k_sb, on_sb = st["q"][b], st["k"][b], st["on"][b]
                expT = [[], []]
                for jt in range(NT):
                    pe_pair = [ps.tile([128, N], F32, tag="pe", bufs=2,
                                       name=f"pe{b}_{hp}_{jt}_{hh}")
                               for hh in range(2)]
                    for ic in range(NCH):
                        for hh in range(2):
                            nc.tensor.matmul(
                                pe_pair[hh][:, ic * 512:(ic + 1) * 512],
                                lhsT=k_sb[hp][hh * 64:(hh + 1) * 64,
                                              jt * 128:(jt + 1) * 128],
                                rhs=q_sb[hp][hh * 64:(hh + 1) * 64,
                                             ic * 512:(ic + 1) * 512],
                                start=True, stop=True,
                            )
                    for hh in range(2):
                        e = sb.tile([128, N], BF16, tag="exp", bufs=8,
                                    name=f"e{b}_{hp}_{jt}_{hh}")
                        nc.scalar.activation(e, pe_pair[hh], AF.Exp, scale=SCALE)
                        expT[hh].append(e)
                for hh in range(2):
                    h = 2 * hp + hh
                    ct, half = divmod(h, 2)
                    pus = [ps.tile([128, 512], F32, tag="pu", bufs=2,
                                   name=f"pu{b}_{h}_{ic}")
                           for ic in range(NCH)]
                    for jt in range(NT):
                        for ic in range(NCH):
                            nc.tensor.matmul(
                                pus[ic][0:HD + 1, :],
                                lhsT=st["vext"][(b, jt)][:, :, h],
                                rhs=expT[hh][jt][:, ic * 512:(ic + 1) * 512],
                                start=(jt == 0), stop=(jt == NT - 1),
                            )
                    for ic in range(NCH):
                        pu = pus[ic]
                        den = sb.tile([1, 512], F32R, tag="den", bufs=2,
                                      name=f"den{b}_{h}_{ic}")
                        nc.vector.tensor_copy(den, pu[HD:HD + 1, :])
                        rb = ps.tile([HD, 512], F32, tag="pq", bufs=2,
                                     name=f"rb{b}_{h}_{ic}")
                        nc.tensor.matmul(rb, lhsT=_r(ones1), rhs=_r(den),
                                         start=True, stop=True)
                        r_sb = sb.tile([HD, 512], F32, tag="rsb", bufs=2,
                                       name=f"r{b}_{h}_{ic}")
                        nc.vector.reciprocal_approx_fast(out=r_sb, in_=rb)
                        nc.vector.tensor_tensor(
                            on_sb[ct][half * 64:(half + 1) * 64,
                                      ic * 512:(ic + 1) * 512],
                            pu[0:HD, :], r_sb, ALU.mult)

            def outproj_group(b, ot, nch):
                p = ps.tile([128, 512], F32, tag="pq", bufs=2,
                            name=f"po{b}{ot}{nch}")
                for ctt in range(CT):
                    nc.tensor.matmul(
                        p,
                        lhsT=w_sb["woT"][ctt][:, ot * 128:(ot + 1) * 128],
                        rhs=st["on"][b][ctt][:, nch * 512:(nch + 1) * 512],
                        start=(ctt == 0), stop=(ctt == CT - 1),
                    )
                yt = sb.tile([128, 512], F32, tag="y", bufs=2,
                             name=f"y{b}{ot}{nch}")
                nc.vector.tensor_scalar(
                    yt, p, bo_sb[:, ot:ot + 1], gam_sb[:, 0:1],
                    ALU.add, ALU.mult)
                nc.vector.tensor_tensor(
                    yt, yt,
                    st["xf"][b][ot][:, nch * 512:(nch + 1) * 512].bitcast(F32),
                    ALU.add)
                nc.gpsimd.dma_start(
                    out=y2[b, ot * 128:(ot + 1) * 128,
                           nch * 512:(nch + 1) * 512],
                    in_=yt)

            # ================= emission schedule =================
            for ot in range(CT):
                proj_qk_group(0, "wqT", ot)
                proj_qk_group(0, "wkT", ot)
            for nt in range(NT):
                proj_v_group(0, nt)
            load_x(1)
            alloc_on(0)
            alloc_on(1)

            # batch0 attention with batch1 projections interleaved
            b1_work = ([("qk", "wqT", ot) for ot in range(CT)]
                       + [("qk", "wkT", ot) for ot in range(CT)]
                       + [("v", nt, 0) for nt in range(NT)])
            for hp in range(HEADS // 2):
                attention_pair(0, hp)
                for item in b1_work[hp * 4:(hp + 1) * 4]:
                    if item[0] == "qk":
                        proj_qk_group(1, item[1], item[2])
                    else:
                        proj_v_group(1, item[1])

            # batch1 attention with batch0 out-projection interleaved
            for hp in range(HEADS // 2):
                attention_pair(1, hp)
                if hp < 2:
                    for i in range(4):
                        g = hp * 4 + i
                        outproj_group(0, g // 2, g % 2)

            for ot in range(CT):
                for nch in range(NCH):
                    outproj_group(1, ot, nch)
    nc.compile()
    return nc


_PROGRAM = None


def _get_program():
    global _PROGRAM
    if _PROGRAM is None:
        _PROGRAM = build_program()
    return _PROGRAM


def kernel(**inputs):
    x = np.ascontiguousarray(inputs["x"], dtype=np.float32)
    B, c, H, W = x.shape
    assert (c, H * W) == (C, N)
    xr = x.reshape(B, C, N)

    wqT = np.ascontiguousarray(inputs["wq"].T.astype(np.float32))
    wkT = np.ascontiguousarray(inputs["wk"].T.astype(np.float32))
    wvT = np.ascontiguousarray(inputs["wv"].T.astype(np.float32))
    woT = np.ascontiguousarray(inputs["wo"].T.astype(np.float32))
    bq_r = np.ascontiguousarray(inputs["bq"].astype(np.float32).reshape(CT, 128).T)
    bk_r = np.ascontiguousarray(inputs["bk"].astype(np.float32).reshape(CT, 128).T)
    bo_r = np.ascontiguousarray(inputs["bo"].astype(np.float32).reshape(CT, 128).T)
    bv = np.ascontiguousarray(inputs["bv"].astype(np.float32))
    gamma = np.ascontiguousarray(inputs["gamma"].astype(np.float32))

    shared = dict(wqT=wqT, wkT=wkT, wvT=wvT, woT=woT,
                  bq_r=bq_r, bk_r=bk_r, bo_r=bo_r, bv=bv, gamma=gamma,
                  ones64=np.ones(HD, dtype=np.float32),
                  ones64h=np.ones(HD, dtype=ml_dtypes.bfloat16))
    in_maps = []
    for core in range(NCORES):
        m = dict(shared)
        m["x2"] = np.ascontiguousarray(xr[core * BPC:(core + 1) * BPC])
        in_maps.append(m)

    nc = _get_program()
    res = run_bass_kernel_spmd(nc, in_maps, list(range(NCORES)))
    y = np.concatenate([res.results[i]["y2"] for i in range(NCORES)], axis=0)
    return y.reshape(B, C, H, W).astype(np.float32)


if __name__ == "__main__":
    rng = np.random.default_rng(0)
    ins = {
        "x": rng.standard_normal((16, C, 32, 32), dtype=np.float32),
        "wq": rng.standard_normal((C, C), dtype=np.float32) / 23,
        "bq": rng.standard_normal((C,), dtype=np.float32) / 23,
        "wk": rng.standard_normal((C, C), dtype=np.float32) / 23,
        "bk": rng.standard_normal((C,), dtype=np.float32) / 23,
        "wv": rng.standard_normal((C, C), dtype=np.float32) / 23,
        "bv": rng.standard_normal((C,), dtype=np.float32) / 23,
        "wo": rng.standard_normal((C, C), dtype=np.float32) / 23,
        "bo": rng.standard_normal((C,), dtype=np.float32) / 23,
        "gamma": np.full((1,), 0.1, dtype=np.float32),
    }
    out = kernel(**ins)
    print("kernel ran, out shape", out.shape)
